# revision 41
# baseline (speedup 1.0000x reference)
"""AttentionStreamBlock on 8 trn2 NeuronCores.

Sharding: core c = (batch b = c//2, sequence half = c%2). Each core computes
its half's 1024 tokens end-to-end; k/v projections are computed for the full
sequence on both cores of a pair (duplicated — cheaper than an AllGather of
k/v); the sequential scan's carry state (513 floats) is exchanged pair-wise
with a tiny AllGather, and the second sequence half re-runs its local scan
seeded with the received carry.

On-chip layout is transposed: [channels, tokens]. The host packs each core's
OWN half at token columns [0:1024]; key order within attention is a
permutation, which softmax attention is invariant to.
"""
import sys
sys.path.insert(0, '/opt/trn_rl_repo')
import os
import numpy as np
import ml_dtypes

B, S, D, H = 4, 2048, 512, 8
T = S // 2
N_CORES = 8
BF = ml_dtypes.bfloat16

_CACHE = {}


from contextlib import ExitStack

import concourse.bass as bass
import concourse.tile as tile
from concourse import mybir

F32 = mybir.dt.float32
BF16 = mybir.dt.bfloat16
F32R = mybir.dt.float32r
F8 = mybir.dt.float8e4
AF = mybir.ActivationFunctionType
OP = mybir.AluOpType
DR = mybir.MatmulPerfMode.DoubleRow
F8NP = ml_dtypes.float8_e4m3
WS = 16.0          # fp8 weight scale (keeps 0.02-scale weights out of subnormals)

D = 512
S = 2048
T = 1024
H = 8
DH = 64
DF = 2048
C = D // 128      # 4
KC = S // 128     # 16
HC = DF // 128    # 16
EPS = 1e-5


def r(ap):
    return ap.bitcast(F32R)


def emit_kernel(nc, with_collective=True, debug_taps=False, loop_n=0):
    ext_f32 = lambda n, s: nc.dram_tensor(n, s, F32, kind="ExternalInput")
    ext_bf = lambda n, s: nc.dram_tensor(n, s, BF16, kind="ExternalInput")
    # sh = mu*x + (1-mu)*x_prev precomputed on host: f32 own half for the
    # P3 residual, fp8 full-S (DR d-order rows) for the qkv matmuls.
    sht = ext_f32("sht", [D, T])
    sh8b = nc.dram_tensor("sh8b", [D, S], F8, kind="ExternalInput")
    w_qkv8 = nc.dram_tensor("w_qkv8", [D, 3 * D], F8, kind="ExternalInput")
    b_qk = ext_f32("b_qk", [128, 8])
    w_o8 = nc.dram_tensor("w_o8", [D, D], F8, kind="ExternalInput")
    ob = ext_f32("ob", [128, 4])
    g1b1 = ext_f32("g1b1", [128, 8])
    tmod = ext_f32("tmod", [128, 4])
    w_rT = ext_bf("w_rT", [D, D])
    w_vvT = ext_bf("w_vvT", [D, D])
    b_rv = ext_f32("b_rv", [128, 8])
    gscbsc = ext_f32("gscbsc", [128, 8])
    g2b2 = ext_f32("g2b2", [128, 8])
    w18 = nc.dram_tensor("w18", [D, DF], F8, kind="ExternalInput")
    b1f = ext_f32("b1f", [128, 16])
    w28 = nc.dram_tensor("w28", [DF, D], F8, kind="ExternalInput")
    b2f = ext_f32("b2f", [128, 4])
    g3b3 = ext_f32("g3b3", [128, 8])
    cmask = ext_f32("cmask", [128, 1])
    outT = nc.dram_tensor("outT", [D, T], F32, kind="ExternalOutput")
    taps = {}
    if debug_taps:
        for tn in ("kTd", "qTd", "aoTd", "x1Td", "bscand", "mergedd", "x2Td"):
            shp = [128, C, S] if tn == "kTd" else [128, C, T]
            taps[tn] = nc.dram_tensor(tn, shp, F32, kind="ExternalOutput")

    with ExitStack() as ctx:
        tc = ctx.enter_context(tile.TileContext(nc))
        if loop_n:
            ctx.enter_context(tc.For_i(0, loop_n, 1))
        const = ctx.enter_context(tc.tile_pool(name="const", bufs=1))
        wts = ctx.enter_context(tc.tile_pool(name="wts", bufs=1))
        act = ctx.enter_context(tc.tile_pool(name="act", bufs=1))
        tmp = ctx.enter_context(tc.tile_pool(name="tmp", bufs=1))
        dram = ctx.enter_context(tc.tile_pool(name="dram", bufs=1, space="DRAM"))

        dma = nc.sync.dma_start
        V = nc.vector
        G = nc.gpsimd

        def sb(i):
            return V if i % 2 == 0 else G

        def t4(n, dtype=F32):
            return tmp.tile([128, T], dtype, tag="t4", bufs=2, name=n)

        def t2(n, dtype=F32):
            return tmp.tile([128, 512], dtype, tag="t2", bufs=4, name=n)

        # ---------------- small constants ----------------
        def small(t_ext, shape):
            tl = const.tile(shape, F32, tag=t_ext.name, name=t_ext.name + "_t")
            dma(tl[:], t_ext[:])
            return tl

        b_qk_t = small(b_qk, [128, 8]); ob_t = small(ob, [128, 4])
        g1b1_t = small(g1b1, [128, 8]); tm_t = small(tmod, [128, 4])
        b_rv_t = small(b_rv, [128, 8]); gsc_t = small(gscbsc, [128, 8])
        g2b2_t = small(g2b2, [128, 8]); b1f_t = small(b1f, [128, 16])
        b2f_t = small(b2f, [128, 4]); g3b3_t = small(g3b3, [128, 8])
        cm_t = small(cmask, [128, 1])

        eps_t = const.tile([128, 1], F32, tag="eps_t")
        V.memset(eps_t[:], EPS * D * D)   # D²-scaled LN variance (see ln_stats_h)
        ones128 = const.tile([128, 128], F32, tag="ones128")
        V.memset(ones128[:], 1.0)
        ones128b = const.tile([128, 128], BF16, tag="ones128b")
        V.memset(ones128b[:], 1.0)
        # 'a'-chain rows are bf16: per-token scale errors cancel in sm_ln
        dm_row = const.tile([1, T], BF16, tag="dm_row")
        se_row = const.tile([1, T], BF16, tag="se_row")
        a0m_t = const.tile([1, 1], F32, tag="a0m_t")

        # ---------------- resident weights (fp8 pairs / bf16) ----------------
        # fp8 DoubleRow layout: [k, j, t, out] where contraction dim index
        # d = j*256 + t*128 + k; weights pre-scaled by WS on host.
        # Queue spreading (only SP/Act/gpsimd can DMA): P1's sh8 chunks get
        # the scalar queue to themselves; w_qkv+w28 on gpsimd; the rest
        # (needed late) on sync. P1 compute can start ~3us in.
        w_qkv8_t = wts.tile([128, 2, 2, 3 * D], F8, tag="w_qkv")
        for j in range(2):
            for t in range(2):
                nc.gpsimd.dma_start(w_qkv8_t[:, j, t, :],
                                    w_qkv8[(2 * j + t) * 128:(2 * j + t + 1) * 128, :])
        w_o8_t = wts.tile([128, 2, 2, D], F8, tag="w_oT")
        for j in range(2):
            for t in range(2):
                dma(w_o8_t[:, j, t, :], w_o8[(2 * j + t) * 128:(2 * j + t + 1) * 128, :])
        w_rT_t = wts.tile([128, C, D], BF16, tag="w_rT")
        for c in range(C):
            dma(w_rT_t[:, c, :], w_rT[c * 128:(c + 1) * 128, :])
        w_vvT_t = wts.tile([128, C, D], BF16, tag="w_vvT")
        for c in range(C):
            dma(w_vvT_t[:, c, :], w_vvT[c * 128:(c + 1) * 128, :])
        w18_t = wts.tile([128, 2, 2, DF], F8, tag="w1T")
        for j in range(2):
            for t in range(2):
                dma(w18_t[:, j, t, :],
                    w18[(2 * j + t) * 128:(2 * j + t + 1) * 128, :])
        w28_t = wts.tile([128, HC // 2, 2, D], F8, tag="w2T")
        for g in range(HC // 2):
            for t in range(2):
                nc.gpsimd.dma_start(w28_t[:, g, t, :],
                                    w28[(2 * g + t) * 128:(2 * g + t + 1) * 128, :])
        # ---------------- P1: q,k,v projections ----------------
        # v is stored fp8 (x WS) in DR pair layout [kcp, t, h, 80]: the pad
        # column holds WS so den scales match (ao8 = av/den cancels WS); 80B
        # slot stride satisfies the DR 16B-alignment rule.
        kT = act.tile([128, C, S], BF16, tag="sA")
        qT = act.tile([128, C, T], BF16, tag="sD")
        vpad = act.tile([128, KC // 2, 2, H, 80], F8, tag="sB")
        V.memset(vpad[:, :, :, :, DH:DH + 1], WS)

        with tc.tile_pool(name="psA", bufs=1, space="PSUM") as psA:
            for st in range(S // 512):
                sh8 = tmp.tile([128, 2, 2, 512], F8, tag="sh", bufs=3,
                               name=f"sh8_{st}")
                xs = slice(st * 512, (st + 1) * 512)
                for j in range(2):
                    for t in range(2):
                        nc.scalar.dma_start(
                            sh8[:, j, t, :],
                            sh8b[(2 * j + t) * 128:(2 * j + t + 1) * 128, xs])
                for dc in range(C):
                    pk = psA.tile([128, 512], F32, tag="pgen", bufs=3, name=f"pk{st}{dc}")
                    for j in range(2):
                        nc.tensor.matmul(pk[:],
                                         w_qkv8_t[:, j, :, D + dc * 128:D + (dc + 1) * 128],
                                         sh8[:, j, :, :], start=(j == 0), stop=(j == 1),
                                         perf_mode=DR)
                    nc.scalar.activation(kT[:, dc, st * 512:(st + 1) * 512], pk[:],
                                         AF.Identity, bias=b_qk_t[:, 4 + dc:5 + dc],
                                         scale=1.0 / WS)
                    if st < T // 512:
                        pq = psA.tile([128, 512], F32, tag="pgen", bufs=3,
                                      name=f"pq{st}{dc}")
                        for j in range(2):
                            nc.tensor.matmul(pq[:],
                                             w_qkv8_t[:, j, :, dc * 128:(dc + 1) * 128],
                                             sh8[:, j, :, :], start=(j == 0), stop=(j == 1),
                                             perf_mode=DR)
                        nc.scalar.activation(qT[:, dc, st * 512:(st + 1) * 512], pq[:],
                                             AF.Identity, bias=b_qk_t[:, dc:dc + 1],
                                             scale=1.0 / WS)
                for sub in range(4):
                    kc = st * 4 + sub
                    pv = psA.tile([128, 512], F32, tag="pgen", bufs=3, name=f"pv{kc}")
                    for j in range(2):
                        nc.tensor.matmul(pv[:], sh8[:, j, :, sub * 128:(sub + 1) * 128],
                                         w_qkv8_t[:, j, :, 2 * D:3 * D],
                                         start=(j == 0), stop=(j == 1), perf_mode=DR)
                    # pv = WS * v_true already; store fp8 at WS scale
                    V.tensor_copy(vpad[:, kc // 2, kc % 2, :, 0:DH],
                                  pv[:].rearrange("p (h c) -> p h c", h=H))

        # ---------------- P2: attention ----------------
        # ao8 [k, j, t, tok]: out-proj contraction dim d = j*256+t*128+k (fp8)
        # et in fp8 e4m3: exp split 3 ways (ACT true-exp, DVE/Pool int8
        # bitcast PLF: e4m3(2^y) ~ int8(round(8y + 56 - 5.5/16))); av
        # accumulated with fp8 DoubleRow over kc pairs (halved PE time).
        ao8 = act.tile([128, 2, 2, T], F8, tag="sE", name="ao8")
        A8 = 8.0 / 0.6931471805599453 * 0.125   # 8/ln2 * score scale
        B8 = 56.0 - 5.5 / 16.0
        I8 = mybir.dt.int8
        with tc.tile_pool(name="psB", bufs=1, space="PSUM") as psB:
            avs = {}

            def finalize(h):
                # Deferred per-head normalization (emitted during the NEXT
                # head's kcp loop so the PE broadcast doesn't stall scores).
                # 1/den broadcast via K=1 ones matmul — Pool's ISA
                # partition_broadcast held its sequencer ~9us per call.
                av = avs.pop(h)
                hp = h % 2
                hr = slice(hp * 64, (hp + 1) * 64)
                den_t = tmp.tile([1, T], F32, tag="den_t", bufs=1,
                                 name=f"den{h}")
                nc.scalar.copy(den_t[:], av[DH:DH + 1, :])
                rrow_b = tmp.tile([1, T], BF16, tag="den_b", bufs=1,
                                  name=f"rrb{h}")
                # custom-DVE recip must read SBUF (PSUM-sourced gave garbage)
                V.reciprocal_approx_fast(den_t[:], den_t[:])
                V.tensor_copy(rrow_b[:], den_t[:])
                rbs_ps = psB.tile([64, T], F32, tag="sc", bufs=2,
                                  name=f"rbsp{h}")
                for qt in range(2):
                    qs = slice(qt * 512, (qt + 1) * 512)
                    nc.tensor.matmul(rbs_ps[:, qs], ones128b[0:1, 0:DH],
                                     rrow_b[:, qs], start=True, stop=True)
                rbs = tmp.tile([64, T], F32, tag="rbs", bufs=1, name=f"rbs{h}")
                nc.scalar.copy(rbs[:], rbs_ps[:])
                V.tensor_tensor(ao8[hr, h // 4, (h // 2) % 2, :], av[0:DH, :],
                                rbs[:], OP.mult)

            for h in range(H):
                hcc, hp = divmod(h, 2)
                hr = slice(hp * 64, (hp + 1) * 64)
                av = psB.tile([DH + 1, T], F32, tag="av", bufs=2, name=f"av{h}")
                avs[h] = av
                for kcp in range(KC // 2):
                    et8 = tmp.tile([128, 2, T], F8, tag="exp", bufs=4,
                                   name=f"et{h}_{kcp}")
                    for t in range(2):
                        kc = 2 * kcp + t
                        sc = psB.tile([128, T], F32, tag="sc", bufs=2,
                                      name=f"sc{h}_{kc}")
                        for qt in range(2):
                            qs = slice(qt * 512, (qt + 1) * 512)
                            nc.tensor.matmul(sc[:, qs],
                                             kT[hr, hcc, kc * 128:(kc + 1) * 128],
                                             qT[hr, hcc, qs], start=True, stop=True)
                        # Pool can't read PSUM: exp splits ACT 9 / DVE 7
                        if t == 0 or kcp == 0:
                            nc.scalar.activation(et8[:, t, :], sc[:], AF.Exp,
                                                 scale=0.125)
                        else:
                            V.tensor_scalar(et8[:, t, :].bitcast(I8), sc[:],
                                            A8, B8, OP.mult, OP.add)
                    for qt in range(2):
                        qs = slice(qt * 512, (qt + 1) * 512)
                        nc.tensor.matmul(av[:, qs], vpad[:, kcp, :, h, 0:DH + 1],
                                         et8[:, :, qs], start=(kcp == 0),
                                         stop=(kcp == KC // 2 - 1), perf_mode=DR)
                    if kcp == 1 and h > 0:
                        finalize(h - 1)
            finalize(H - 1)

        def tap(name, tile_ap, cdim=C, width=T):
            if debug_taps:
                for c in range(cdim):
                    for w0 in range(0, width, 512):
                        tpc = tmp.tile([128, 512], F32, tag="tapt", bufs=1, name=f"tap{name}{c}_{w0}")
                        V.tensor_copy(tpc[:], tile_ap[:, c, w0:w0 + 512])
                        dma(taps[name][:, c, w0:w0 + 512], tpc[:])

        if debug_taps:
            tap("kTd", kT, width=S)
            tap("qTd", qT)

        psC = ctx.enter_context(tc.tile_pool(name="psC", bufs=1, space="PSUM"))

        def pgen(n):
            return psC.tile([128, 512], F32, tag="pgen", bufs=3, name=n)

        def pstat(n, shape=None):
            return psC.tile(shape or [128, 512], F32, tag="stat", bufs=3, name=n)

        # ---------------- LN helpers ----------------
        def t2f(n, dtype=F32, w=512):
            return tmp.tile([128, w], dtype, tag="t2s", bufs=7, name=n)

        def ln_stats_h(x_ap_of, qs, ra=None, nm=""):
            """Per-token LN stats over a token slice qs, D²-scaled:
            returns (mean_b, A_b/D); the 1/D is absorbed into host-side
            gains (g·D) / downstream weights. The mean hop is off the
            critical path: D²·var = D·Q − S² straight from PSUM."""
            w = qs.stop - qs.start
            Sp = pstat(f"S_{nm}", [128, w])
            Qp = pstat(f"Q_{nm}", [128, w])
            for c in range(C):
                xa = x_ap_of(c)[:, qs]
                sq = t2f(f"sq_{nm}{c}", BF16, w)
                sb(c).tensor_tensor(sq[:], xa, xa, OP.mult)
                nc.tensor.matmul(Sp[:], r(ones128[:]), r(xa),
                                 start=(c == 0), stop=(c == C - 1))
                nc.tensor.matmul(Qp[:], ones128b[:], sq[:],
                                 start=(c == 0), stop=(c == C - 1))
            mean = tmp.tile([128, w], F32, tag="lnM", bufs=3, name=f"mean_{nm}")
            nc.scalar.activation(mean[:], Sp[:], AF.Copy, scale=1.0 / D)
            msq = t2f(f"msq_{nm}", F32, w)
            nc.scalar.activation(msq[:], Sp[:], AF.Square)
            var = t2f(f"var_{nm}", F32, w)
            V.scalar_tensor_tensor(var[:], Qp[:], float(D), msq[:],
                                   OP.mult, OP.subtract)
            if ra is not None:
                ra2 = t2f(f"ra2_{nm}", F32, w)
                G.tensor_tensor(ra2[:], ra[:, qs], ra[:, qs], OP.mult)
                var2 = t2f(f"var2_{nm}", F32, w)
                V.tensor_tensor(var2[:], var[:], ra2[:], OP.mult)
                var = var2
            sd = t2f(f"sd_{nm}", F32, w)
            nc.scalar.activation(sd[:], var[:], AF.Sqrt, bias=eps_t[:])
            A_b = tmp.tile([128, w], F32, tag="lnA", bufs=3, name=f"A_{nm}")
            V.reciprocal_approx_fast(A_b[:], sd[:])
            if ra is not None:
                Ar = tmp.tile([128, w], F32, tag="lnA", bufs=3, name=f"Ar_{nm}")
                V.tensor_tensor(Ar[:], A_b[:], ra[:, qs], OP.mult)
                A_b = Ar
            return mean, A_b

        def ln_apply_h(x_ap, mean, A_b, out_ap, c, nm="", g=None, b=None,
                       w=512):
            t1 = t2f(f"apl1_{nm}{c}", F32, w)
            sb(c).tensor_tensor(t1[:], x_ap, mean[:], OP.subtract)
            if g is None:
                sb(c + 1).tensor_tensor(out_ap, t1[:], A_b[:], OP.mult)
            else:
                tb = t2f(f"apl2_{nm}{c}", F32, w)
                sb(c + 1).tensor_tensor(tb[:], t1[:], A_b[:], OP.mult)
                nc.scalar.activation(out_ap, tb[:], AF.Identity,
                                     bias=b, scale=g)

        def staged(stats_fn, apply_fn, n):
            MAs = [None] * n
            MAs[0] = stats_fn(0)
            for q in range(1, n):
                MAs[q] = stats_fn(q)
                apply_fn(q - 1, *MAs[q - 1])
            apply_fn(n - 1, *MAs[n - 1])

        # ---------------- P3: out-proj + residual + ln1 ----------------
        x1T = act.tile([128, C, T], F32, tag="sF")
        for dc in range(C):
            for qt in range(2):
                qs = slice(qt * 512, (qt + 1) * 512)
                sh_t = t2(f"sh2_{dc}{qt}")
                dma(sh_t[:], sht[dc * 128:(dc + 1) * 128, qs])
                pt = pgen(f"po_{dc}{qt}")
                for j in range(2):
                    nc.tensor.matmul(pt[:], w_o8_t[:, j, :, dc * 128:(dc + 1) * 128],
                                     ao8[:, j, :, qs], start=(j == 0), stop=(j == 1),
                                     perf_mode=DR)
                V.scalar_tensor_tensor(r(x1T[:, dc, qs]), pt[:], ob_t[:, dc:dc + 1],
                                       sh_t[:], OP.add, OP.add)
        QS4 = [slice(0, 512), slice(512, 1024)]

        def l1_stats(q):
            return ln_stats_h(lambda c: x1T[:, c, :], QS4[q], nm=f"l1{q}")

        def l1_apply(q, M1, A1):
            for c in range(C):
                ln_apply_h(x1T[:, c, QS4[q]], M1, A1, r(x1T[:, c, QS4[q]]),
                           c, f"l1{q}", g=g1b1_t[:, c:c + 1],
                           b=g1b1_t[:, 4 + c:5 + c], w=512)

        for ht in range(2):
            l1_apply(ht, *l1_stats(ht))

        if debug_taps:
            tap("x1Td", x1T)

        # ---------------- P4: sequence-merging scan ----------------
        dec_all = act.tile([128, C, T], BF16, tag="sE", name="dec_all")
        u_all = act.tile([128, C, T], BF16, tag="sG", name="u_all")
        bscan = act.tile([128, C, T], F32, tag="sD", name="bscan")
        for c in range(C):
            nc.scalar.activation(dec_all[:, c, :], x1T[:, c, :], AF.Sigmoid,
                                 scale=tm_t[:, c:c + 1])
        dmPs = [pstat(f"dmP{ht}", [1, 512]) for ht in range(2)]
        sePs = [pstat(f"seP{ht}", [1, 512]) for ht in range(2)]
        if with_collective:
            st_in = dram.tile([1, 516], F32)
            st_out = dram.tile([2, 516], F32)
        for c in range(C):
            e_t = t4(f"e_{c}", BF16)
            nc.scalar.activation(e_t[:], x1T[:, c, :], AF.Exp)
            sb(c).tensor_tensor(u_all[:, c, :], e_t[:], x1T[:, c, :], OP.mult)
            if with_collective:
                V.tensor_tensor_scan(r(bscan[:, c, :]), dec_all[:, c, :], u_all[:, c, :],
                                     0.0, OP.mult, OP.add)
                dma(st_in[0:1, c * 128:(c + 1) * 128].rearrange("o p -> p o"),
                    bscan[:, c, T - 1:T])
            for ht in range(2):
                qs = slice(ht * 512, (ht + 1) * 512)
                nc.tensor.matmul(dmPs[ht][0:1, :], ones128b[:, 0:1], dec_all[:, c, qs],
                                 start=(c == 0), stop=(c == C - 1))
                nc.tensor.matmul(sePs[ht][0:1, :], ones128b[:, 0:1], e_t[:, qs],
                                 start=(c == 0), stop=(c == C - 1))
        for ht in range(2):
            qs = slice(ht * 512, (ht + 1) * 512)
            V.tensor_scalar(dm_row[:, qs], dmPs[ht][:], 1.0 / D, None, OP.mult)
            V.tensor_copy(se_row[:, qs], sePs[ht][:])
        b0m = const.tile([128, C], F32, tag="b0m")
        if with_collective:
            a1_row = tmp.tile([1, T], F32, tag="a_row", bufs=1, name="a1_row")
            V.tensor_tensor_scan(a1_row[:], dm_row[:], se_row[:],
                                 0.0, OP.mult, OP.add)
            dma(st_in[0:1, 512:513], a1_row[:, T - 1:T])
            nc.gpsimd.collective_compute(
                "AllGather", OP.bypass, ins=[st_in.opt()], outs=[st_out.opt()],
                replica_groups=[[0, 1], [2, 3], [4, 5], [6, 7]])
            for c in range(C):
                b0c = tmp.tile([128, 1], F32, tag="b0c", bufs=2, name=f"b0c{c}")
                dma(b0c[:], st_out[0:1, c * 128:(c + 1) * 128].rearrange("o p -> p o"))
                V.tensor_scalar(b0m[:, c:c + 1], b0c[:], cm_t[:], None, OP.mult)
            a0_t = tmp.tile([1, 1], F32, tag="a0_t", bufs=1)
            dma(a0_t[:], st_out[0:1, 512:513])
            V.tensor_scalar(a0m_t[:], a0_t[:], cm_t[0:1, :], None, OP.mult)
        else:
            V.memset(b0m[:], 0.0)
            V.memset(a0m_t[:], 0.0)

        for c in range(C):
            V.tensor_tensor_scan(r(bscan[:, c, 0:512]), dec_all[:, c, 0:512],
                                 u_all[:, c, 0:512], b0m[:, c:c + 1],
                                 OP.mult, OP.add)
            V.tensor_tensor_scan(r(bscan[:, c, 512:T]), dec_all[:, c, 512:T],
                                 u_all[:, c, 512:T], bscan[:, c, 511:512],
                                 OP.mult, OP.add)
        a2_row = tmp.tile([1, T], F32, tag="a_row", bufs=1, name="a2_row")
        V.tensor_tensor_scan(a2_row[:], dm_row[:], se_row[:],
                             a0m_t[:], OP.mult, OP.add)
        V.tensor_scalar(a2_row[:], a2_row[:], 1e-8, None, OP.add)
        V.reciprocal_approx_fast(a2_row[:], a2_row[:])
        a2r_b = tmp.tile([1, T], BF16, tag="a_row2", bufs=1, name="a2rb")
        nc.scalar.copy(a2r_b[:], a2_row[:])
        ra_b = tmp.tile([128, T], F32, tag="ra_b", bufs=1)
        for ht in range(2):
            qs = slice(ht * 512, (ht + 1) * 512)
            ra_ps = pstat(f"ra_ps{ht}")
            nc.tensor.matmul(ra_ps[:], ones128b[0:1, :], a2r_b[:, qs],
                             start=True, stop=True)
            nc.scalar.copy(ra_b[:, qs], ra_ps[:])

        # ---------------- P5/P6: per-half pipelined tail ----------------
        merged = act.tile([128, C, T], BF16, tag="sG", name="merged")
        x2T = act.tile([128, C, T], F32, tag="sB", name="x2T")
        x2f8 = act.tile([128, 2, 2, T], F8, tag="sE", name="x2f8")
        x3pre = act.tile([128, C, T], F32, tag="sA", name="x3pre")
        # Stages are emitted ht-zipped so the two independent half-streams
        # overlap: while one half's LN-stat chain waits, engines run the other.
        QS = [slice(0, 512), slice(512, 1024)]

        def m_stats(q):
            return ln_stats_h(lambda c: bscan[:, c, :], QS4[q], ra=ra_b,
                              nm=f"m{q}")

        def m_apply(q, Mm, Am):
            for c in range(C):
                ln_apply_h(bscan[:, c, QS4[q]], Mm, Am,
                           merged[:, c, QS4[q]], c, f"m{q}", w=512)

        staged(m_stats, m_apply, 2)
        for ht in range(2):
            qs = QS[ht]
            for dc in range(C):
                pr = pgen(f"pr_{dc}{ht}")
                for di in range(C):
                    nc.tensor.matmul(pr[:], w_rT_t[:, di, dc * 128:(dc + 1) * 128],
                                     merged[:, di, qs], start=(di == 0), stop=(di == C - 1))
                sig = tmp.tile([128, 512], F32, tag="sv", bufs=4, name=f"sig_{dc}{ht}")
                nc.scalar.activation(sig[:], pr[:], AF.Sigmoid,
                                     bias=b_rv_t[:, dc:dc + 1])
                pv = pgen(f"pvv_{dc}{ht}")
                for di in range(C):
                    nc.tensor.matmul(pv[:], w_vvT_t[:, di, dc * 128:(dc + 1) * 128],
                                     merged[:, di, qs], start=(di == 0), stop=(di == C - 1))
                V.scalar_tensor_tensor(r(x2T[:, dc, qs]), pv[:],
                                       b_rv_t[:, 4 + dc:5 + dc], sig[:],
                                       OP.add, OP.mult)
        def sc_stats(q):
            return ln_stats_h(lambda c: x2T[:, c, :], QS4[q], nm=f"sc{q}")

        def sc_apply(q, Msc, Asc):
            qs = QS4[q]
            for c in range(C):
                t1 = t2f(f"cpa_{c}{q}", F32, 512)
                sb(c).tensor_tensor(t1[:], x2T[:, c, qs], Msc[:], OP.subtract)
                tb = t2f(f"cpb_{c}{q}", F32, 512)
                sb(c + 1).tensor_tensor(tb[:], t1[:], Asc[:], OP.mult)
                t3 = t2f(f"cpc_{c}{q}", F32, 512)
                V.scalar_tensor_tensor(t3[:], tb[:], gsc_t[:, c:c + 1],
                                       x1T[:, c, qs], OP.mult, OP.add)
                nc.scalar.activation(r(x2T[:, c, qs]), t3[:], AF.Identity,
                                     bias=gsc_t[:, 4 + c:5 + c])

        staged(sc_stats, sc_apply, 2)

        def l2_stats(q):
            return ln_stats_h(lambda c: x2T[:, c, :], QS4[q], nm=f"l2{q}")

        def l2_apply(q, M2, A2):
            qs = QS4[q]
            for c in range(C):
                ln_apply_h(x2T[:, c, qs], M2, A2, r(x2T[:, c, qs]), c,
                           f"l2{q}", g=g2b2_t[:, c:c + 1],
                           b=g2b2_t[:, 4 + c:5 + c], w=512)
                nc.scalar.copy(x2f8[:, c // 2, c % 2, qs], x2T[:, c, qs])

        staged(l2_stats, l2_apply, 2)

        # FFN (fused h -> out), fp8 DoubleRow; w1/w2 pre-scaled ×WS on
        # host, h stored as h/WS so the w2 product needs no unscale.
        def ffn(ht):
            # Two dc-pair passes over a 2-bank accumulator (frees PSUM for
            # deeper pgen/stat rotation); h2 tiles stay resident for pass B.
            qs = QS[ht]
            h2s = []
            fa0 = psC.tile([128, 1024], F32, tag="facc", bufs=1, name=f"facc{ht}_0")
            for g in range(HC // 2):
                h2 = tmp.tile([128, 2, 512], F8, tag="h_t", bufs=8,
                              name=f"h{ht}_{g}")
                h2s.append(h2)
                for t in range(2):
                    hcx = 2 * g + t
                    ph = pgen(f"ph_{ht}{hcx}")
                    for j in range(2):
                        nc.tensor.matmul(ph[:], w18_t[:, j, :, hcx * 128:(hcx + 1) * 128],
                                         x2f8[:, j, :, qs], start=(j == 0), stop=(j == 1),
                                         perf_mode=DR)
                    nc.scalar.activation(h2[:, t, :], ph[:], AF.Relu,
                                         bias=b1f_t[:, hcx:hcx + 1],
                                         scale=1.0 / (WS * WS))
                for dc in range(2):
                    nc.tensor.matmul(fa0[:, dc * 512:(dc + 1) * 512],
                                     w28_t[:, g, :, dc * 128:(dc + 1) * 128],
                                     h2[:, :, :], start=(g == 0), stop=(g == HC // 2 - 1),
                                     perf_mode=DR)
            for dc in range(2):
                V.scalar_tensor_tensor(r(x3pre[:, dc, qs]), fa0[:, dc * 512:(dc + 1) * 512],
                                       b2f_t[:, dc:dc + 1], x2T[:, dc, qs],
                                       OP.add, OP.add)
            fa1 = psC.tile([128, 1024], F32, tag="facc", bufs=1, name=f"facc{ht}_1")
            for g in range(HC // 2):
                for dc in range(2):
                    nc.tensor.matmul(fa1[:, dc * 512:(dc + 1) * 512],
                                     w28_t[:, g, :, (2 + dc) * 128:(3 + dc) * 128],
                                     h2s[g][:, :, :], start=(g == 0), stop=(g == HC // 2 - 1),
                                     perf_mode=DR)
            for dc in range(2, C):
                V.scalar_tensor_tensor(r(x3pre[:, dc, qs]),
                                       fa1[:, (dc - 2) * 512:(dc - 1) * 512],
                                       b2f_t[:, dc:dc + 1], x2T[:, dc, qs],
                                       OP.add, OP.add)

        def l3_out(ht, M3, A3):
            qs = QS[ht]
            for c in range(C):
                o_t = t2f(f"o_{c}{ht}")
                ln_apply_h(x3pre[:, c, qs], M3, A3, o_t[:], c, f"l3{ht}",
                           g=g3b3_t[:, c:c + 1], b=g3b3_t[:, 4 + c:5 + c])
                dma(outT[c * 128:(c + 1) * 128, qs], o_t[:])

        ffn(0)
        MA30 = ln_stats_h(lambda c: x3pre[:, c, :], QS[0], nm="l30")
        ffn(1)
        l3_out(0, MA30[0], MA30[1])
        MA31 = ln_stats_h(lambda c: x3pre[:, c, :], QS[1], nm="l31")
        l3_out(1, MA31[0], MA31[1])

        if debug_taps:
            tap("bscand", bscan)
            tap("mergedd", merged)
            tap("x2Td", x2T)

    return nc



def _build():
    if "nc" in _CACHE:
        return _CACHE["nc"]
    from concourse import bacc
    nc = bacc.Bacc("TRN2", target_bir_lowering=False, debug=False,
                   num_devices=N_CORES)
    emit_kernel(nc, with_collective=True)
    nc.compile()
    _CACHE["nc"] = nc
    return nc


def _chunks(v):
    """[512] -> [128, 4] chunk-major columns."""
    return np.ascontiguousarray(np.asarray(v, np.float32).reshape(-1, 128).T)


def _to8(a):
    """Transpose-to-[in,out], scale by WS, clip to TRN e4m3 range, cast."""
    return np.ascontiguousarray(
        np.clip(np.asarray(a, np.float32).T * WS, -240.0, 240.0)).astype(F8NP)


def prep_core_inputs(inputs):
    f = {k: np.asarray(v, np.float32) for k, v in inputs.items()}
    mu = float(f["mu"])
    w_qkv8 = _to8(f["in_proj_w"])
    b_qk = _chunks(f["in_proj_b"][0:2 * D].reshape(-1))
    w_o8 = np.ascontiguousarray(
        np.clip(f["out_w"].T, -240.0, 240.0)).astype(F8NP)
    ob = _chunks(f["out_w"] @ f["in_proj_b"][2 * D:3 * D] + f["out_b"])
    # LN gains absorb the ×D of the D²-scaled on-device rstd (see ln_stats_h)
    g1b1 = np.concatenate([_chunks(f["ln1_g"] * D), _chunks(f["ln1_b"])], axis=1)
    tmod = _chunks(f["time_mod"])
    wr_f = f["wr_w"] * f["sm_ln_g"][None, :] * D
    wv_f = f["wv_w"] * f["sm_ln_g"][None, :] * D
    b_r = f["wr_b"] + f["wr_w"] @ f["sm_ln_b"]
    b_vv = f["wv_b"] + f["wv_w"] @ f["sm_ln_b"]
    w_rT = np.ascontiguousarray(wr_f.T).astype(BF)
    w_vvT = np.ascontiguousarray(wv_f.T).astype(BF)
    b_rv = np.concatenate([_chunks(b_r), _chunks(b_vv)], axis=1)
    gscbsc = np.concatenate([_chunks(f["sc_ln_g"] * D), _chunks(f["sc_ln_b"])], axis=1)
    g2b2 = np.concatenate([_chunks(f["ln2_g"] * D), _chunks(f["ln2_b"])], axis=1)
    w18 = _to8(f["w1"])
    b1f = _chunks(f["b1"] / WS)
    w28 = _to8(f["w2"])
    b2f = _chunks(f["b2"])
    g3b3 = np.concatenate([_chunks(f["ln3_g"] * D), _chunks(f["ln3_b"])], axis=1)

    shared = dict(w_qkv8=w_qkv8, b_qk=b_qk, w_o8=w_o8, ob=ob, g1b1=g1b1,
                  tmod=tmod, w_rT=w_rT, w_vvT=w_vvT, b_rv=b_rv, gscbsc=gscbsc,
                  g2b2=g2b2, w18=w18, b1f=b1f, w28=w28, b2f=b2f, g3b3=g3b3)

    in_maps = []
    for b in range(B):
        # sh = mu*x + (1-mu)*x_prev computed on host, [D, S]
        shT = np.asarray(mu * f["x"][b] + (1.0 - mu) * f["x_prev"][b]).T
        sh8 = np.clip(shT, -240, 240).astype(F8NP)
        for half in range(2):
            own = slice(half * T, (half + 1) * T)
            oth = slice((1 - half) * T, (2 - half) * T)
            m = dict(shared)
            m["sht"] = np.ascontiguousarray(shT[:, own])
            m["sh8b"] = np.ascontiguousarray(
                np.concatenate([sh8[:, own], sh8[:, oth]], axis=1))
            m["cmask"] = np.full((128, 1), float(half), np.float32)
            in_maps.append(m)
    return in_maps


def run_spmd(in_maps):
    from concourse.bass_utils import run_bass_kernel_spmd
    nc = _build()
    return run_bass_kernel_spmd(nc, in_maps, list(range(N_CORES)))


def time_spmd(in_maps, chain=6, reps=3):
    """Time steady-state per-execution latency by chaining the NEFF `chain`
    times inside one jit (outputs feed the next call's donated buffers)."""
    import time
    import jax
    import jax.numpy as jnp
    from jax.sharding import Mesh, PartitionSpec
    from jax.experimental.shard_map import shard_map
    from concourse import bass2jax
    from concourse.bass2jax import _bass_exec_p, install_neuronx_cc_hook
    from concourse import mybir
    nc = _build()
    install_neuronx_cc_hook()

    pname = nc.partition_id_tensor.name if nc.partition_id_tensor else None
    in_names, out_names, out_avals, zero_outs = [], [], [], []
    for alloc in nc.m.functions[0].allocations:
        if not isinstance(alloc, mybir.MemoryLocationSet):
            continue
        name = alloc.memorylocations[0].name
        if alloc.kind == "ExternalInput":
            if name != pname:
                in_names.append(name)
        elif alloc.kind == "ExternalOutput":
            out_names.append(name)
            shape = tuple(alloc.tensor_shape)
            dtype = mybir.dt.np(alloc.dtype)
            out_avals.append(jax.core.ShapedArray(shape, dtype))
            zero_outs.append(np.zeros(shape, dtype))
    n_params = len(in_names)
    all_names = in_names + out_names
    if pname is not None:
        all_names = all_names + [pname]

    def one(args):
        if pname is not None:
            args = args + [bass2jax.partition_id_tensor()]
        outs = _bass_exec_p.bind(
            *args,
            out_avals=tuple(out_avals),
            in_names=tuple(all_names),
            out_names=tuple(out_names),
            lowering_input_output_aliases=(),
            sim_require_finite=True, sim_require_nnan=True, nc=nc)
        return tuple(outs)

    def body_chain(*args):
        ins = list(args[:n_params])
        outs = list(args[n_params:])
        for _ in range(chain):
            outs = list(one(ins + outs))
        return tuple(outs)

    def body_single(*args):
        return one(list(args))

    devices = jax.devices()[:N_CORES]
    mesh = Mesh(np.array(devices), ("core",))
    n_outs = len(out_names)
    donate = tuple(range(n_params, n_params + n_outs))

    def compile_fn(body):
        return jax.jit(shard_map(body, mesh=mesh,
                                 in_specs=(PartitionSpec("core"),) * (n_params + n_outs),
                                 out_specs=(PartitionSpec("core"),) * n_outs,
                                 check_rep=False),
                       donate_argnums=donate, keep_unused=True)

    per_core = [[np.asarray(m[nm]) for nm in in_names] for m in in_maps]
    concat_in = [np.concatenate([per_core[c][i] for c in range(N_CORES)], axis=0)
                 for i in range(n_params)]

    fn = compile_fn(body_single)
    ts = []
    din = [jax.device_put(x) for x in concat_in]
    zs = [np.zeros((N_CORES * z.shape[0], *z.shape[1:]), z.dtype)
          for z in zero_outs]
    out = fn(*din, *zs)
    jax.block_until_ready(out)
    for rep in range(reps + 6):
        t0 = time.time()
        out = fn(*din, *out)   # donate previous outputs as buffers
        jax.block_until_ready(out)
        ts.append(time.time() - t0)
    best = min(ts)
    print("singles (ms):", [f"{t*1e3:.2f}" for t in ts])
    return best


def kernel(**inputs) -> np.ndarray:
    in_maps = prep_core_inputs(inputs)
    res = run_spmd(in_maps)
    out = np.empty((B, S, D), np.float32)
    for c in range(N_CORES):
        b, half = divmod(c, 2)
        out[b, half * T:(half + 1) * T, :] = res.results[c]["outT"].T
    return out

